# revision 18
# baseline (speedup 1.0000x reference)
"""AttentionDeform TRN2 Bass kernel.

Reference computation (B=1, C=128, H=4, HD=32, N=4096, DIM=3):
  q/k/v = conv1x1(eigen)          -> per-head attention (softmax over keys)
  add_value = wmh @ attn + bmh
  cat = [eigen; add_value] -> conv1x1(2C->2C) -> BN(train) -> ReLU -> conv1x1(2C->C)
  motion = eigen + h;  out = wt @ motion + bt   -> [1, N, 3]

Sharding: 8 cores, each owns a 512-query slice. Every core gets full
eigen (for K/V) + its query slice. Attention stays on-chip in S^T
layout (keys on partitions): softmax denominator comes from a fused
"ones" column in the P@V stationary operand. BN batch stats use a tiny
[128,4] AllReduce across the 8 cores.
"""

import numpy as np

import concourse.bass as bass
import concourse.mybir as mybir
import concourse.tile as tile
from concourse import bacc
from concourse.bass_utils import run_bass_kernel_spmd

N_CORES = 8
C = 128
H = 4
HD = 32
N = 4096
NL = N // N_CORES  # 512 queries per core
DIM = 3
EPS = 1e-5
SCALE = float(1.0 / np.sqrt(np.float32(HD)))

F32 = mybir.dt.float32
F32R = mybir.dt.float32r
AF = mybir.ActivationFunctionType
ALU = mybir.AluOpType


def _build_program():
    nc = bacc.Bacc(
        "TRN2",
        target_bir_lowering=False,
        debug=False,
        num_devices=N_CORES,
    )

    d = {}

    def din(name, shape, dt=F32):
        d[name] = nc.dram_tensor(name, list(shape), dt, kind="ExternalInput").ap()
        return d[name]

    eigen_d = din("eigen", [C, N], F32R)
    eigq_d = din("eigen_q", [C, NL], F32R)
    wqT_d = din("wqT", [C, C], F32R)
    wkT_d = din("wkT", [C, C], F32R)
    wvT_d = din("wvT", [C, C], F32R)
    bq_d = din("bq", [C, 1])
    bk_d = din("bk", [C, 1])
    wmhT_d = din("wmhT", [H, HD, C], F32R)       # head h: wmh.T[32h:32h+32, :]
    bmh2_d = din("bmh2", [C, 1])           # wmh @ bv + bmh
    wc1T_d = din("wc1T", [4, 128, 128], F32R)    # block b=2*i+o: wc1.T[128i:,128o:]
    bc1_d = din("bc1", [128, 2])
    gam_d = din("gamma2", [128, 2])
    bet_d = din("beta2", [128, 2])
    wc2T_d = din("wc2T", [2, 128, 128], F32R)    # block o: wc2.T[128o:, :]
    bc2_d = din("bc2", [C, 1])
    wtT_d = din("wtT", [C, 4], F32R)
    bt_d = din("btr", [1, 4], F32R)
    out_d = nc.dram_tensor("out", [NL, DIM], F32, kind="ExternalOutput").ap()

    with tile.TileContext(nc) as tc:
        with (
            tc.tile_pool(name="consts", bufs=1) as consts,
            tc.tile_pool(name="big", bufs=1) as big,
            tc.tile_pool(name="ppool", bufs=3) as ppool,
            tc.tile_pool(name="work", bufs=2) as work,
            tc.tile_pool(name="spsum", bufs=2, space="PSUM") as spsum,
            tc.tile_pool(name="pvpsum", bufs=2, space="PSUM") as pvpsum,
            tc.tile_pool(name="mpsum", bufs=2, space="PSUM") as mpsum,
            tc.tile_pool(name="dram", bufs=1, space="DRAM") as dram,
        ):
            # ---- constant loads ----
            def load(name, shape, src_ap, dt=F32):
                t = consts.tile(list(shape), dt, tag=name)
                nc.sync.dma_start(t[:], src_ap)
                return t

            eig = load("eig", [C, N], eigen_d[:], F32R)
            eigq = load("eigq", [C, NL], eigq_d[:], F32R)
            wqT = load("wqT", [C, C], wqT_d[:], F32R)
            wkT = load("wkT", [C, C], wkT_d[:], F32R)
            wvT = load("wvT", [C, C], wvT_d[:], F32R)
            bq = load("bq", [C, 1], bq_d[:])
            bk = load("bk", [C, 1], bk_d[:])
            wmhT = load("wmhT", [HD, H, C], wmhT_d[:].rearrange("h p c -> p h c"), F32R)
            bmh2 = load("bmh2", [C, 1], bmh2_d[:])
            wc1T = load("wc1T", [128, 4, 128], wc1T_d[:].rearrange("b p c -> p b c"), F32R)
            bc1 = load("bc1", [128, 2], bc1_d[:])
            gam = load("gam", [128, 2], gam_d[:])
            bet = load("bet", [128, 2], bet_d[:])
            wc2T = load("wc2T", [128, 2, 128], wc2T_d[:].rearrange("b p c -> p b c"), F32R)
            bc2 = load("bc2", [C, 1], bc2_d[:])
            wtT = load("wtT", [C, 4], wtT_d[:], F32R)
            btr = load("btr", [1, 4], bt_d[:], F32R)

            ones = consts.tile([C, 128], F32R, tag="ones")
            nc.vector.memset(ones[:].bitcast(F32), 1.0)
            eps_sb = consts.tile([C, 1], F32, tag="eps")
            nc.vector.memset(eps_sb[:], EPS)

            # heads 0-1 in *a, heads 2-3 in *b — matmul operands may only
            # base at partition 0/32/64, so head 3 can't live at 96..127
            ka = big.tile([64, N], F32R, tag="ka")
            kb = big.tile([64, N], F32R, tag="kb")
            qa = big.tile([64, NL], F32R, tag="qa")
            qb = big.tile([64, NL], F32R, tag="qb")
            # vt layout: [128 part, 32 key-blocks, 4 heads * 33]
            # cols 33h..33h+31 = v^T for head h, col 33h+32 = 1.0
            vt = big.tile([C, 32, 4 * 33], F32R, tag="vt")
            attn_sb = big.tile([32, H, NL], F32R, tag="attn")
            h1_sb = big.tile([128, 2, NL], F32, tag="h1")
            stats = big.tile([128, 4], F32, tag="stats")

            # ones columns of vt (written once)
            nc.vector.memset(
                vt[:].rearrange("p b (h e) -> p b h e", e=33)[:, :, :, 32:33]
                .bitcast(F32),
                1.0,
            )

            # ---- projections ----
            # k = wk @ eigen + bk, split into head-pair halves
            for jc in range(N // 512):
                cs = slice(jc * 512, (jc + 1) * 512)
                for half, dst in ((0, ka), (1, kb)):
                    kp = mpsum.tile([64, 512], F32, tag="m")
                    nc.tensor.matmul(
                        kp[:], wkT[:, half * 64:half * 64 + 64],
                        eig[:, cs], start=True, stop=True,
                    )
                    nc.vector.tensor_scalar_add(
                        dst[:, cs], kp[:], bk[half * 64:half * 64 + 64, :]
                    )
            # q slice = wq @ eigen_q + bq
            for half, dst in ((0, qa), (1, qb)):
                qp = mpsum.tile([64, 512], F32, tag="m")
                nc.tensor.matmul(
                    qp[:], wqT[:, half * 64:half * 64 + 64], eigq[:],
                    start=True, stop=True,
                )
                nc.vector.tensor_scalar_add(
                    dst[:], qp[:], bq[half * 64:half * 64 + 64, :]
                )

            # v^T blocks: vt[n + 128j, c] = v[c, 128j + n]  (no bias; folded
            # into bmh2 = wmh @ bv + bmh)
            for j in range(32):
                vp = mpsum.tile([128, 512], F32, tag="m")
                nc.tensor.matmul(
                    vp[:, 0:128],
                    eig[:, j * 128:(j + 1) * 128],
                    wvT[:],
                    start=True, stop=True,
                )
                nc.vector.tensor_copy(
                    vt[:, j, :].rearrange("p (h e) -> p h e", e=33)[:, :, 0:32],
                    vp[:, 0:128].rearrange("p (h e) -> p h e", e=32),
                )

            # ---- attention (per head), S^T layout: keys on partitions ----
            for h in range(H):
                k_half = ka if h < 2 else kb
                q_half = qa if h < 2 else qb
                hs = slice(32 * (h % 2), 32 * (h % 2) + 32)
                pv = pvpsum.tile([33, NL], F32, tag="pv")
                for jj in range(16):
                    sp = spsum.tile([128, 1024], F32, tag="s")
                    for u in range(2):
                        j = 2 * jj + u
                        nc.tensor.matmul(
                            sp[:, u * 512:(u + 1) * 512],
                            k_half[hs, j * 128:(j + 1) * 128],
                            q_half[hs, :],
                            start=True, stop=True,
                        )
                    p = ppool.tile([128, 1024], F32R, tag="p")
                    # softmax numerator: exp(scale * s); logits are tiny
                    # (|s*scale| < ~3) so no max subtraction is needed
                    nc.scalar.activation(p[:], sp[:], AF.Exp, scale=SCALE)
                    for u in range(2):
                        j = 2 * jj + u
                        nc.tensor.matmul(
                            pv[:],
                            vt[:, j, 33 * h:33 * h + 33],
                            p[:, u * 512:(u + 1) * 512],
                            start=(j == 0), stop=(j == 31),
                        )
                # rows 0..31 = unnormalized attn out; row 32 = softmax denom
                rc = work.tile([33, NL], F32R, tag="rc")
                with nc.allow_low_precision(reason="f32r bits == f32 bits"):
                    nc.vector.reciprocal(rc[32:33, :], pv[32:33, :])
                rb = mpsum.tile([128, 512], F32, tag="m")
                nc.tensor.matmul(
                    rb[0:32, :], ones[32:33, 0:32], rc[32:33, :],
                    start=True, stop=True,
                )
                rbs = work.tile([32, NL], F32, tag="rbs")
                nc.vector.tensor_copy(rbs[:], rb[0:32, :])
                nc.vector.tensor_mul(attn_sb[:, h, :], pv[0:32, :], rbs[:])

            # ---- add_value = wmh @ attn + bmh2 ----
            av = mpsum.tile([128, 512], F32, tag="m")
            for h in range(H):
                nc.tensor.matmul(
                    av[:], wmhT[:, h, :], attn_sb[:, h, :],
                    start=(h == 0), stop=(h == H - 1),
                )
            avs = work.tile([128, NL], F32R, tag="avs")
            nc.vector.tensor_scalar_add(avs[:], av[:], bmh2[:])

            # ---- h1 = wc1 @ [eigen_q; avs] + bc1 ----
            for o in range(2):
                hp = mpsum.tile([128, 512], F32, tag="m")
                for i, rhs in ((0, eigq), (1, avs)):
                    nc.tensor.matmul(
                        hp[:], wc1T[:, 2 * i + o, :], rhs[:],
                        start=(i == 0), stop=(i == 1),
                    )
                nc.vector.tensor_scalar_add(h1_sb[:, o, :], hp[:], bc1[:, o:o + 1])
                # local BN stats: sum and sum of squares over this core's 512
                sq = work.tile([128, NL], F32, tag="sq")
                nc.scalar.activation(
                    sq[:], h1_sb[:, o, :], AF.Square,
                    accum_out=stats[:, 2 * o + 1:2 * o + 2],
                )
                nc.vector.reduce_sum(
                    stats[:, 2 * o:2 * o + 1], h1_sb[:, o, :],
                    axis=mybir.AxisListType.X,
                )

            # ---- global BN stats via AllReduce ----
            stats_in = dram.tile([128, 4], F32, tag="sin")
            stats_out = dram.tile([128, 4], F32, tag="sout")
            nc.sync.dma_start(stats_in[:], stats[:])
            nc.gpsimd.collective_compute(
                "AllReduce",
                ALU.add,
                replica_groups=[list(range(N_CORES))],
                ins=[stats_in.opt()],
                outs=[stats_out.opt()],
            )
            gst = work.tile([128, 4], F32, tag="gst")
            nc.sync.dma_start(gst[:], stats_out[:])

            bn = work.tile([128, 12], F32, tag="bn")
            mean = bn[:, 0:2]
            ex2 = bn[:, 2:4]
            var = bn[:, 4:6]
            std = bn[:, 6:8]
            scl = bn[:, 8:10]
            shf = bn[:, 10:12]
            inv_n = 1.0 / float(N)
            nc.vector.tensor_scalar_mul(mean[:], gst[:, 0:4:2], inv_n)
            nc.vector.tensor_scalar_mul(ex2[:], gst[:, 1:4:2], inv_n)
            # var = E[x^2] - mean^2
            nc.vector.scalar_tensor_tensor(
                var[:], mean[:], -1.0, mean[:], op0=ALU.mult, op1=ALU.mult
            )
            nc.vector.tensor_add(var[:], var[:], ex2[:])
            nc.scalar.activation(std[:], var[:], AF.Sqrt, bias=eps_sb[:])
            nc.vector.reciprocal(std[:], std[:])
            nc.vector.tensor_mul(scl[:], std[:], gam[:])
            # shift = beta - mean * scale
            nc.vector.scalar_tensor_tensor(
                shf[:], mean[:], -1.0, scl[:], op0=ALU.mult, op1=ALU.mult
            )
            nc.vector.tensor_add(shf[:], shf[:], bet[:])

            # ---- h2 = relu(scale*h1 + shift); h3 = wc2 @ h2 + bc2 ----
            mo = mpsum.tile([128, 512], F32, tag="m")
            for o in range(2):
                h2 = work.tile([128, NL], F32R, tag="h2")
                nc.scalar.activation(
                    h2[:], h1_sb[:, o, :], AF.Relu,
                    bias=shf[:, o:o + 1], scale=scl[:, o:o + 1],
                )
                nc.tensor.matmul(
                    mo[:], wc2T[:, o, :], h2[:],
                    start=(o == 0), stop=(o == 1),
                )
            # motion = h3 + bc2 + eigen_q
            motion = work.tile([128, NL], F32R, tag="motion")
            nc.vector.scalar_tensor_tensor(
                motion[:], mo[:], bc2[:], eigq[:], op0=ALU.add, op1=ALU.add
            )

            # ---- out[n, d] = motion^T @ wt^T + bt ----
            for jb in range(NL // 128):
                fo = mpsum.tile([128, 512], F32, tag="m")
                nc.tensor.matmul(
                    fo[:, 0:4],
                    motion[:, jb * 128:(jb + 1) * 128],
                    wtT[:],
                    start=True, stop=False,
                )
                nc.tensor.matmul(
                    fo[:, 0:4], ones[0:1, 0:128], btr[:],
                    start=False, stop=True,
                )
                fos = work.tile([128, DIM], F32, tag="fos")
                nc.vector.tensor_copy(fos[:], fo[:, 0:DIM])
                nc.sync.dma_start(out_d[jb * 128:(jb + 1) * 128, :], fos[:])

    nc.compile()
    return nc


_NC_CACHE = {}


def _get_program():
    if "nc" not in _NC_CACHE:
        _NC_CACHE["nc"] = _build_program()
    return _NC_CACHE["nc"]


def _prep_maps(inputs):
    f = np.float32
    eigen = np.ascontiguousarray(inputs["eigen"].reshape(C, N), dtype=f)
    wq = np.asarray(inputs["wq"], f)
    wk = np.asarray(inputs["wk"], f)
    wv = np.asarray(inputs["wv"], f)
    wmh = np.asarray(inputs["wmh"], f)
    wc1 = np.asarray(inputs["wc1"], f)
    wc2 = np.asarray(inputs["wc2"], f)
    wt = np.asarray(inputs["wt"], f)
    bmh2 = (wmh @ np.asarray(inputs["bv"], f) + np.asarray(inputs["bmh"], f))

    wc1T = np.asarray(inputs["wc1"], f).T  # [256 ci, 256 co]
    wc1T_blocks = np.stack(
        [
            wc1T[128 * i:128 * (i + 1), 128 * o:128 * (o + 1)]
            for i in range(2)
            for o in range(2)
        ]
    )  # b = 2*i + o
    wc2T = wc2.T  # [256, 128]
    wc2T_blocks = np.stack([wc2T[128 * o:128 * (o + 1), :] for o in range(2)])
    wmhT = np.ascontiguousarray(wmh.T.reshape(H, HD, C))

    common = {
        "eigen": eigen,
        "wqT": np.ascontiguousarray(wq.T),
        "wkT": np.ascontiguousarray(wk.T),
        "wvT": np.ascontiguousarray(wv.T),
        "bq": np.asarray(inputs["bq"], f).reshape(C, 1),
        "bk": np.asarray(inputs["bk"], f).reshape(C, 1),
        "wmhT": wmhT,
        "bmh2": bmh2.reshape(C, 1),
        "wc1T": np.ascontiguousarray(wc1T_blocks),
        "bc1": np.ascontiguousarray(
            np.asarray(inputs["bc1"], f).reshape(2, 128).T
        ),
        "gamma2": np.ascontiguousarray(
            np.asarray(inputs["gamma"], f).reshape(2, 128).T
        ),
        "beta2": np.ascontiguousarray(
            np.asarray(inputs["beta"], f).reshape(2, 128).T
        ),
        "wc2T": np.ascontiguousarray(wc2T_blocks),
        "bc2": np.asarray(inputs["bc2"], f).reshape(C, 1),
        "wtT": np.ascontiguousarray(np.pad(wt.T, ((0, 0), (0, 1)))),
        "btr": np.pad(np.asarray(inputs["bt"], f).reshape(1, DIM), ((0, 0), (0, 1))),
    }
    in_maps = []
    for core in range(N_CORES):
        m = dict(common)
        m["eigen_q"] = np.ascontiguousarray(
            eigen[:, core * NL:(core + 1) * NL]
        )
        in_maps.append(m)
    return in_maps


def kernel(**inputs) -> np.ndarray:
    nc = _get_program()
    in_maps = _prep_maps(inputs)
    res = run_bass_kernel_spmd(nc, in_maps, list(range(N_CORES)))
    out = np.concatenate([res.results[c]["out"] for c in range(N_CORES)], axis=0)
    return out.reshape(1, N, DIM)
